# revision 45
# baseline (speedup 1.0000x reference)
"""Multi-head attention (B=2, S=2048, D=1024, H=16) on 8 Trainium2 cores.

Sharding: core c handles batch b = c//4 and head group g = c%4 (4 heads,
256 of the 1024 QKV output columns).

Design (all-bf16 matmuls, fp32 PSUM; ~206 us vs the 221 us baseline):
  - QKV projections stream xT windows of 512 against W.T column blocks;
    q/k psums evict on DVE with bias folded in (tensor_scalar_add) into
    qT/kT [128p (2 heads x 64 hd), blk, S].
  - Attention per (hp, j, i) slot: the two heads' K=64 logits matmuls sit
    on disjoint PE row groups (partitions 0-63 / 64-127) and execute
    CONCURRENTLY (PE row tiling, ~390 ns for the pair); one [128,1024]
    exp on ACT covers both heads (the pacing engine, ~1.01 us/slot); DVE
    mask-multiply with a stride-0 broadcast of the keepT slice; PV
    accumulates with the ones-augmented V (row 64 = softmax denominator).
  - Block (hp0, j0)'s masked exp weights are staged to SBUF and its PV
    is deferred into late-run PE slack (through the proj psum bank), so
    the 16 v projections don't have to crowd the first 16 slots.
  - All other projections ride a deadline-ordered FIFO filler queue,
    emitted between a slot's QK and PV; block drains are split into 3
    thunks consumed one per slot (pv eviction, then per-head: 4 PE
    transposes into one [128,4,65] psum tile, a 4-wide reciprocal of the
    denominators, one broadcast multiply, output DMA) so drain work never
    sits ahead of a seam's QK on the PE queue.
  - Startup: dummy warm-up matmuls hold the PE HAM clock at 2.4 GHz while
    the first DMAs land; the wq/bq/xT-w0-upper/keepT-j0 transfers ride
    the ACT hwdge queue in parallel with wk/bk/xT on the SP queue.
"""

import numpy as np

B, S, D, H = 2, 2048, 1024, 16
HD = D // H  # 64
HEADS_PER_CORE = 4
COLS = HEADS_PER_CORE * HD  # 256
N_CORES = 8
KT = D // 128  # 8 contraction tiles for projections
ST = S // 128  # 16 s tiles
NW = 4  # 512-wide windows
SCALE = 1.0 / np.sqrt(np.float32(D))

_cache = {}


def _build_nc():
    import concourse.bass as bass
    import concourse.mybir as mybir
    import concourse.tile as tile
    from concourse.masks import make_identity

    f32 = mybir.dt.float32
    bf16 = mybir.dt.bfloat16

    nc = bass.Bass(trn_type="TRN2")

    xT = nc.dram_tensor("xT", [D, S], bf16, kind="ExternalInput")
    wq = nc.dram_tensor("wq", [D, COLS], bf16, kind="ExternalInput")
    wk = nc.dram_tensor("wk", [D, COLS], bf16, kind="ExternalInput")
    wv = nc.dram_tensor("wv", [D, COLS], bf16, kind="ExternalInput")
    bq = nc.dram_tensor("bq", [128, 2], f32, kind="ExternalInput")
    bk = nc.dram_tensor("bk", [128, 2], f32, kind="ExternalInput")
    bv = nc.dram_tensor("bv", [1, COLS], bf16, kind="ExternalInput")
    keepT = nc.dram_tensor("keepT", [S, S], bf16, kind="ExternalInput")
    o = nc.dram_tensor("o", [S, COLS], f32, kind="ExternalOutput")

    with tile.TileContext(nc) as tc:
        with (
            tc.tile_pool(name="singles", bufs=1) as singles,
            tc.tile_pool(name="persist", bufs=1) as persist,
            tc.tile_pool(name="big_ps", bufs=2, space="PSUM") as big_ps,
            tc.tile_pool(name="pv_ps", bufs=2, space="PSUM") as pv_ps,
            tc.tile_pool(name="proj_ps", bufs=1, space="PSUM") as proj_ps,
            tc.tile_pool(name="tr_ps", bufs=1, space="PSUM") as tr_ps,
            tc.tile_pool(name="expw", bufs=4) as expw_pool,
            tc.tile_pool(name="expw2", bufs=4) as expw2_pool,
            tc.tile_pool(name="tails", bufs=4) as tails,
        ):
            # ---- constants ----
            ones_col = singles.tile([1, 128], bf16)
            nc.vector.memset(ones_col, 1.0)
            identity = singles.tile([128, 128], f32)
            make_identity(nc, identity)
            id_bf = singles.tile([128, 512], bf16)
            nc.vector.memset(id_bf, 1.0)
            bq_sb = singles.tile([128, 2], f32)
            bk_sb = singles.tile([128, 2], f32)
            bv_sb = singles.tile([1, COLS], bf16)

            # ---- persistent buffers ----
            wq_sb = persist.tile([128, KT, COLS], bf16)
            wk_sb = persist.tile([128, KT, COLS], bf16)
            wv_sb = persist.tile([128, KT, COLS], bf16)
            xT_sb = persist.tile([128, KT, S], bf16)
            keepT_sb = persist.tile([128, ST, S], bf16)
            qT_sb = persist.tile([128, 2, S], bf16)
            kT_sb = persist.tile([128, 2, S], bf16)
            v_aug = persist.tile([128, ST, HEADS_PER_CORE, HD + 1], bf16)
            nc.vector.memset(v_aug[:, :, :, HD : HD + 1], 1.0)
            # masked exp weights of block (hp0, j0) staged here so its PV
            # (which needs all of v) can run late, in PE slack, instead of
            # forcing every v projection into the first 16 slots.
            stage = persist.tile([128, ST, 1024], bf16)

            # ---- DMA issue (order = priority) ----
            xT_r = xT[:, :].rearrange("(kt p) s -> p kt s", p=128)
            keepT_r = keepT[:, :].rearrange("(i p) s -> p i s", p=128)

            def dma_xT_w(w):
                for h in range(2):
                    nc.sync.dma_start(
                        out=xT_sb[:, 4 * h : 4 * h + 4, w * 512 : (w + 1) * 512],
                        in_=xT_r[:, 4 * h : 4 * h + 4, w * 512 : (w + 1) * 512],
                    )

            def dma_keep_j(j, i0, i1, eng):
                eng.dma_start(
                    out=keepT_sb[:, i0:i1, j * 512 : (j + 1) * 512],
                    in_=keepT_r[:, i0:i1, j * 512 : (j + 1) * 512],
                )

            def dma_w(w_sb, w_dram, eng):
                eng.dma_start(
                    out=w_sb,
                    in_=w_dram[:, :].rearrange("(kt p) c -> p kt c", p=128),
                )

            # Two hwdge queues run concurrently; balance payloads against
            # consumption order. The k/q projection chain gates the first
            # exp, so its inputs (wk, wq, biases, xT w0 split across both
            # queues) come first everywhere.
            # ACT queue: wq + bq + xT w0 upper half + the j0 keep block.
            dma_w(wq_sb, wq, nc.scalar)
            nc.scalar.dma_start(out=bq_sb, in_=bq[:, :])
            nc.scalar.dma_start(
                out=xT_sb[:, 4:8, 0:512], in_=xT_r[:, 4:8, 0:512]
            )
            dma_keep_j(0, 0, 8, nc.scalar)
            dma_keep_j(0, 8, 16, nc.scalar)
            # SP queue: wk + bk + xT w0 lower half, remaining xT windows,
            # then the later keep blocks.
            dma_w(wk_sb, wk, nc.sync)
            nc.sync.dma_start(out=bk_sb, in_=bk[:, :])
            nc.sync.dma_start(
                out=xT_sb[:, 0:4, 0:512], in_=xT_r[:, 0:4, 0:512]
            )
            dma_w(wv_sb, wv, nc.sync)
            nc.sync.dma_start(out=bv_sb, in_=bv[:, :])
            dma_xT_w(1)
            dma_xT_w(2)
            dma_xT_w(3)
            dma_keep_j(1, 0, 16, nc.sync)
            dma_keep_j(2, 0, 16, nc.sync)
            dma_keep_j(3, 0, 16, nc.sync)
            # fmt: off

            # ---- PE warm-up: garbage matmuls on the identity tile keep the
            # HAM activity window busy while the first DMAs land, so the
            # first projection runs at 2.4 GHz instead of 1.2.
            warm = proj_ps.tile([128, 512], f32, tag="proj")
            for r in range(20):
                nc.tensor.matmul(
                    warm,
                    lhsT=id_bf[:, 0:128],
                    rhs=id_bf[:, :],
                    start=(r == 0),
                    stop=False,
                    skip_group_check=True,
                )
            # second warm-up stage gated on the wk DMA, bridging the HAM
            # window until the first projection's xT window lands
            for r in range(10):
                nc.tensor.matmul(
                    warm,
                    lhsT=wk_sb[:, 0, 0:128],
                    rhs=id_bf[:, :],
                    start=False,
                    stop=(r == 9),
                    skip_group_check=True,
                )

            # ---- projection groups (each split into 2 filler units so a
            # single unit never exceeds a slot's PE slack) ----
            _proj_state = {}

            def proj_qk_part(which, blk, w, part):
                w_sb, b_sb, dst = (
                    (wq_sb, bq_sb, qT_sb),
                    (wk_sb, bk_sb, kT_sb),
                )[which]
                if part == 0:
                    ps = proj_ps.tile([128, 512], f32, tag="proj")
                    _proj_state[(which, blk, w)] = ps
                else:
                    ps = _proj_state.pop((which, blk, w))
                for kt in range(4 * part, 4 * part + 4):
                    nc.tensor.matmul(
                        ps,
                        lhsT=w_sb[:, kt, blk * 128 : (blk + 1) * 128],
                        rhs=xT_sb[:, kt, w * 512 : (w + 1) * 512],
                        start=(kt == 0),
                        stop=(kt == KT - 1),
                        skip_group_check=True,
                    )
                if part == 1:
                    nc.vector.tensor_scalar_add(
                        out=dst[:, blk, w * 512 : (w + 1) * 512],
                        in0=ps,
                        scalar1=b_sb[:, blk : blk + 1],
                    )

            def proj_qk(which, blk, w):
                proj_qk_part(which, blk, w, 0)
                proj_qk_part(which, blk, w, 1)

            def proj_v_part(st, part):
                if part == 0:
                    psv = proj_ps.tile([128, COLS], f32, tag="proj")
                    _proj_state[("v", st)] = psv
                    nc.tensor.matmul(
                        psv,
                        lhsT=ones_col[:, :],
                        rhs=bv_sb[:, :],
                        start=True,
                        stop=False,
                        skip_group_check=True,
                    )
                else:
                    psv = _proj_state.pop(("v", st))
                for kt in range(4 * part, 4 * part + 4):
                    nc.tensor.matmul(
                        psv,
                        lhsT=xT_sb[:, kt, st * 128 : (st + 1) * 128],
                        rhs=wv_sb[:, kt, :],
                        start=False,
                        stop=(kt == KT - 1),
                        skip_group_check=True,
                    )
                if part == 1:
                    nc.vector.tensor_copy(
                        out=v_aug[:, st, :, 0:HD],
                        in_=psv.rearrange("p (h d) -> p h d", h=HEADS_PER_CORE),
                    )

            # Filler queue: (deadline_slot, thunk), consumed FIFO — the list
            # is constructed in non-decreasing deadline order with each proj
            # group's two halves adjacent (the single-buffer proj psum pool
            # requires a group to finish before the next starts). Fillers
            # run after the slot's QK and before its PV.
            def u_qk(which, blk, w, part):
                return lambda: proj_qk_part(which, blk, w, part)

            def u_v(st, part):
                return lambda: proj_v_part(st, part)

            _pvj0 = {}

            def u_pvj0(e, part):
                def go():
                    if part == 0:
                        psJ = proj_ps.tile([HD + 1, 512], f32, tag="proj")
                        _pvj0[e] = psJ
                    else:
                        psJ = _pvj0[e]
                    for i in range(4 * part, 4 * part + 4):
                        nc.tensor.matmul(
                            psJ,
                            lhsT=v_aug[:, i, e, :],
                            rhs=stage[:, i, e * 512 : (e + 1) * 512],
                            start=(i == 0),
                            stop=(i == ST - 1),
                            skip_group_check=True,
                        )
                    if part == 3:
                        pv_sb = tails.tile(
                            [HD + 1, 512], f32, tag="pvsb", name=f"pvj0sb{e}"
                        )
                        nc.vector.tensor_copy(out=pv_sb, in_=psJ)
                        _pvj0[(e, "sb")] = pv_sb

                return go

            def u_normj0(e):
                def go():
                    norm_head(0, 0, e, _pvj0.pop((e, "sb")))

                return go

            qk_units = lambda dl, which, blk, w: [
                (dl, u_qk(which, blk, w, 0)),
                (dl, u_qk(which, blk, w, 1)),
            ]
            v_units = lambda dl, st: [(dl, u_v(st, 0)), (dl, u_v(st, 1))]
            def qk_split(d0, d1, which, blk, w):
                return [
                    (d0, u_qk(which, blk, w, 0)),
                    (d1, u_qk(which, blk, w, 1)),
                ]

            fillers = []
            fillers += v_units(1, 0)
            fillers += qk_split(2, 3, 1, 0, 1)  # k blk0 w1 (QK i=4 at slot 4)
            fillers += v_units(3, 1) + v_units(4, 2) + v_units(5, 3)
            fillers += qk_split(6, 7, 1, 0, 2)
            fillers += v_units(7, 4) + v_units(8, 5) + v_units(9, 6)
            fillers += qk_split(10, 11, 1, 0, 3)
            fillers += v_units(11, 7)
            fillers += qk_split(12, 13, 0, 0, 1)  # q blk0 w1 (j1 at slot 16)
            fillers += v_units(13, 8) + v_units(14, 9) + v_units(15, 10)
            fillers += v_units(16, 11) + v_units(17, 12) + v_units(18, 13)
            fillers += v_units(19, 14) + v_units(20, 15)
            fillers += qk_split(28, 29, 0, 0, 2)  # q blk0 w2 (due <= 31)
            fillers += qk_split(33, 34, 1, 1, 0)  # k blk1 (due <= 63)
            fillers += qk_split(35, 36, 1, 1, 1)
            fillers += qk_split(37, 38, 1, 1, 2)
            fillers += qk_split(39, 40, 1, 1, 3)
            fillers += qk_split(44, 45, 0, 0, 3)  # q blk0 w3 (due <= 47)
            fillers += qk_split(48, 49, 0, 1, 0)  # q blk1 w0 (due <= 63)
            fillers += qk_split(50, 51, 0, 1, 1)
            fillers += qk_split(52, 53, 0, 1, 2)
            fillers += qk_split(54, 55, 0, 1, 3)
            # deferred PV of (hp0, j0) through the freed proj psum bank,
            # riding the fillerless hp1-era slack
            fillers += [(68 + 2 * p, u_pvj0(0, p)) for p in range(4)]
            fillers += [(76 + 2 * p, u_pvj0(1, p)) for p in range(4)]
            fillers += [(86, u_normj0(0)), (88, u_normj0(1))]

            def norm_head(hp, j, e, pv_sb):
                h = 2 * hp + e
                ob = tails.tile([128, 4, HD], f32, tag="ob")
                tr = tr_ps.tile([128, 4, HD + 1], f32, tag="tr")
                for c in range(4):
                    nc.tensor.transpose(
                        out=tr[:, c, :],
                        in_=pv_sb[:, c * 128 : (c + 1) * 128],
                        identity=identity[0 : HD + 1, 0 : HD + 1],
                    )
                rc = tails.tile([128, 4], f32, tag="rc")
                nc.vector.reciprocal(out=rc, in_=tr[:, :, HD : HD + 1])
                rc_ap = rc[:, :]
                rc_bcast = bass.AP(
                    tensor=rc_ap.tensor,
                    offset=rc_ap.offset,
                    ap=[*rc_ap.ap, [0, HD]],
                )
                nc.vector.tensor_mul(out=ob, in0=tr[:, :, 0:HD], in1=rc_bcast)
                nc.sync.dma_start(
                    out=o[
                        j * 512 : (j + 1) * 512, h * HD : (h + 1) * HD
                    ].rearrange("(c p) d -> p c d", p=128),
                    in_=ob,
                )

            def drain_thunks(hp, j, pvs):
                """Split the block drain into 3 thunks consumed one per
                slot, so drain work never sits ahead of a seam's QK on the
                PE queue."""
                pv_sbs = []

                def evict():
                    for e in range(2):
                        pv_sb = tails.tile(
                            [HD + 1, 512], f32, tag="pvsb", name=f"pv_sb{e}"
                        )
                        nc.vector.tensor_copy(out=pv_sb, in_=pvs[e])
                        pv_sbs.append(pv_sb)

                def norm(e):
                    return lambda: norm_head(hp, j, e, pv_sbs[e])

                return [evict, norm(0), norm(1)]

            # Preamble: first k/q windows so attention starts immediately.
            proj_qk(1, 0, 0)  # k blk0 w0
            proj_qk(0, 0, 0)  # q blk0 w0

            drainq = []
            pvs = None
            for slot in range(2 * NW * ST):
                hp, rem = divmod(slot, NW * ST)
                j, i = divmod(rem, ST)
                # QK + exp + mask first: keeps ACT fed across block seams.
                lgp = big_ps.tile([128, 1024], f32, tag="big")
                for e in range(2):
                    po = e * 64
                    nc.tensor.matmul(
                        lgp[:, e * 512 : (e + 1) * 512],
                        lhsT=kT_sb[po : po + 64, hp, i * 128 : (i + 1) * 128],
                        rhs=qT_sb[po : po + 64, hp, j * 512 : (j + 1) * 512],
                        start=True,
                        stop=True,
                        skip_group_check=True,
                    )
                ex = expw_pool.tile([128, 1024], bf16)
                nc.scalar.activation(
                    out=ex,
                    in_=lgp,
                    func=mybir.ActivationFunctionType.Exp,
                    scale=float(SCALE),
                )
                defer = slot < ST  # (hp0, j0): PV deferred via `stage`
                if defer:
                    ex2 = stage[:, i, :]
                else:
                    ex2 = expw2_pool.tile([128, 1024], bf16)
                k_ap = keepT_sb[:, i, j * 512 : (j + 1) * 512]
                k_bcast = bass.AP(
                    tensor=k_ap.tensor,
                    offset=k_ap.offset,
                    ap=[k_ap.ap[0], [0, 2], *k_ap.ap[1:]],
                )
                nc.vector.tensor_mul(
                    out=ex2.rearrange("p (e n) -> p e n", e=2),
                    in0=ex.rearrange("p (e n) -> p e n", e=2),
                    in1=k_bcast,
                )
                if drainq:
                    drainq.pop(0)()
                while fillers and fillers[0][0] <= slot:
                    fillers.pop(0)[1]()
                if not defer:
                    if i == 0:
                        pvs = [
                            pv_ps.tile(
                                [HD + 1, 512], f32, tag="pv", name=f"pv{e}"
                            )
                            for e in range(2)
                        ]
                    for e in range(2):
                        nc.tensor.matmul(
                            pvs[e],
                            lhsT=v_aug[:, i, 2 * hp + e, :],
                            rhs=ex2[:, e * 512 : (e + 1) * 512],
                            start=(i == 0),
                            stop=(i == ST - 1),
                            skip_group_check=True,
                        )
                    if i == ST - 1:
                        drainq.extend(drain_thunks(hp, j, pvs))
            while drainq:
                drainq.pop(0)()

    _split_multiwait(nc)
    return nc


def _split_multiwait(nc, max_waits: int = 1):
    import concourse.mybir as mybir

    for f in nc.m.functions:
        for blk in f.blocks:
            out = []
            changed = False
            for inst in blk.instructions:
                si = inst.sync_info
                if si is not None and len(si.on_wait) > max_waits:
                    waits = list(si.on_wait)
                    extra = waits[: len(waits) - max_waits]
                    keep = waits[len(waits) - max_waits :]
                    for k, w in enumerate(extra):
                        out.append(
                            mybir.InstNoOp(
                                name=f"{inst.name}-wfx{k}",
                                engine=inst.engine,
                                sync_info=mybir.SyncInfo(on_wait=[w], on_update=[]),
                                bass_nofuse=True,
                            )
                        )
                    inst.sync_info = mybir.SyncInfo(
                        on_wait=keep, on_update=list(si.on_update)
                    )
                    changed = True
                out.append(inst)
            if changed:
                blk.instructions = out
    return nc


def _prep_in_maps(x, mask, Wq, bq, Wk, bk, Wv, bv):
    import ml_dtypes

    bf16 = ml_dtypes.bfloat16
    x = np.asarray(x, np.float32)
    mask = np.asarray(mask, bool)

    xT_b = [np.ascontiguousarray(x[b].T).astype(bf16) for b in range(B)]
    keepT_b = [
        np.ascontiguousarray((~mask[b, 0]).T).astype(bf16) for b in range(B)
    ]
    WqT = np.asarray(Wq, np.float32).T.astype(bf16)
    WkT = np.asarray(Wk, np.float32).T.astype(bf16)
    WvT = np.asarray(Wv, np.float32).T.astype(bf16)
    bq32 = np.asarray(bq, np.float32)
    bk32 = np.asarray(bk, np.float32)
    bv = np.asarray(bv, np.float32).astype(bf16)

    in_maps = []
    for c in range(N_CORES):
        b, g = divmod(c, 4)
        cols = slice(g * COLS, (g + 1) * COLS)
        in_maps.append(
            {
                "xT": xT_b[b],
                "wq": np.ascontiguousarray(WqT[:, cols]),
                "wk": np.ascontiguousarray(WkT[:, cols]),
                "wv": np.ascontiguousarray(WvT[:, cols]),
                "bq": np.ascontiguousarray(bq32[cols].reshape(2, 128).T),
                "bk": np.ascontiguousarray(bk32[cols].reshape(2, 128).T),
                "bv": np.ascontiguousarray(bv[cols].reshape(1, COLS)),
                "keepT": keepT_b[b],
            }
        )
    return in_maps


def kernel(x, mask, Wq, bq, Wk, bk, Wv, bv, _trace=False):
    from concourse.bass_utils import run_bass_kernel_spmd

    if "nc" not in _cache:
        _cache["nc"] = _build_nc()
    nc = _cache["nc"]

    in_maps = _prep_in_maps(x, mask, Wq, bq, Wk, bk, Wv, bv)
    res = run_bass_kernel_spmd(
        nc, in_maps, core_ids=list(range(N_CORES)), trace=_trace
    )
    _cache["last_result"] = res

    out = np.empty((B, S, D), np.float32)
    for c in range(N_CORES):
        b, g = divmod(c, 4)
        out[b, :, g * COLS : (g + 1) * COLS] = res.results[c]["o"]
    return out


# revision 46
# speedup vs baseline: 1.0067x; 1.0067x over previous
"""Multi-head attention (B=2, S=2048, D=1024, H=16) on 8 Trainium2 cores.

Sharding: core c handles batch b = c//4 and head group g = c%4 (4 heads,
256 of the 1024 QKV output columns).

Design (all-bf16 matmuls, fp32 PSUM; ~206 us vs the 221 us baseline):
  - QKV projections stream xT windows of 512 against W.T column blocks;
    q/k psums evict on DVE with bias folded in (tensor_scalar_add) into
    qT/kT [128p (2 heads x 64 hd), blk, S].
  - Attention per (hp, j, i) slot: the two heads' K=64 logits matmuls sit
    on disjoint PE row groups (partitions 0-63 / 64-127) and execute
    CONCURRENTLY (PE row tiling, ~390 ns for the pair); one [128,1024]
    exp on ACT covers both heads (the pacing engine, ~1.01 us/slot); DVE
    mask-multiply with a stride-0 broadcast of the keepT slice; PV
    accumulates with the ones-augmented V (row 64 = softmax denominator).
  - Block (hp0, j0)'s masked exp weights are staged to SBUF and its PV
    is deferred into late-run PE slack (through the proj psum bank), so
    the 16 v projections don't have to crowd the first 16 slots.
  - All other projections ride a deadline-ordered FIFO filler queue,
    emitted between a slot's QK and PV; block drains are split into 3
    thunks consumed one per slot (pv eviction, then per-head: 4 PE
    transposes into one [128,4,65] psum tile, a 4-wide reciprocal of the
    denominators, one broadcast multiply, output DMA) so drain work never
    sits ahead of a seam's QK on the PE queue.
  - Startup: dummy warm-up matmuls hold the PE HAM clock at 2.4 GHz while
    the first DMAs land; the wq/bq/xT-w0-upper/keepT-j0 transfers ride
    the ACT hwdge queue in parallel with wk/bk/xT on the SP queue.
"""

import numpy as np

B, S, D, H = 2, 2048, 1024, 16
HD = D // H  # 64
HEADS_PER_CORE = 4
COLS = HEADS_PER_CORE * HD  # 256
N_CORES = 8
KT = D // 128  # 8 contraction tiles for projections
ST = S // 128  # 16 s tiles
NW = 4  # 512-wide windows
SCALE = 1.0 / np.sqrt(np.float32(D))

_cache = {}


def _build_nc():
    import concourse.bass as bass
    import concourse.mybir as mybir
    import concourse.tile as tile
    from concourse.masks import make_identity

    f32 = mybir.dt.float32
    bf16 = mybir.dt.bfloat16

    nc = bass.Bass(trn_type="TRN2")

    xT = nc.dram_tensor("xT", [D, S], bf16, kind="ExternalInput")
    wq = nc.dram_tensor("wq", [D, COLS], bf16, kind="ExternalInput")
    wk = nc.dram_tensor("wk", [D, COLS], bf16, kind="ExternalInput")
    wv = nc.dram_tensor("wv", [D, COLS], bf16, kind="ExternalInput")
    bq = nc.dram_tensor("bq", [128, 2], f32, kind="ExternalInput")
    bk = nc.dram_tensor("bk", [128, 2], f32, kind="ExternalInput")
    bv = nc.dram_tensor("bv", [1, COLS], bf16, kind="ExternalInput")
    keepT = nc.dram_tensor("keepT", [S, S], bf16, kind="ExternalInput")
    o = nc.dram_tensor("o", [S, COLS], f32, kind="ExternalOutput")

    with tile.TileContext(nc) as tc:
        with (
            tc.tile_pool(name="singles", bufs=1) as singles,
            tc.tile_pool(name="persist", bufs=1) as persist,
            tc.tile_pool(name="big_ps", bufs=2, space="PSUM") as big_ps,
            tc.tile_pool(name="pv_ps", bufs=2, space="PSUM") as pv_ps,
            tc.tile_pool(name="proj_ps", bufs=1, space="PSUM") as proj_ps,
            tc.tile_pool(name="tr_ps", bufs=1, space="PSUM") as tr_ps,
            tc.tile_pool(name="expw", bufs=4) as expw_pool,
            tc.tile_pool(name="expw2", bufs=4) as expw2_pool,
            tc.tile_pool(name="tails", bufs=4) as tails,
        ):
            # ---- constants ----
            ones_col = singles.tile([1, 128], bf16)
            nc.vector.memset(ones_col, 1.0)
            identity = singles.tile([128, 128], f32)
            make_identity(nc, identity)
            id_bf = singles.tile([128, 512], bf16)
            nc.vector.memset(id_bf, 1.0)
            bq_sb = singles.tile([128, 2], f32)
            bk_sb = singles.tile([128, 2], f32)
            bv_sb = singles.tile([1, COLS], bf16)

            # ---- persistent buffers ----
            wq_sb = persist.tile([128, KT, COLS], bf16)
            wk_sb = persist.tile([128, KT, COLS], bf16)
            wv_sb = persist.tile([128, KT, COLS], bf16)
            xT_sb = persist.tile([128, KT, S], bf16)
            keepT_sb = persist.tile([128, ST, S], bf16)
            qT_sb = persist.tile([128, 2, S], bf16)
            kT_sb = persist.tile([128, 2, S], bf16)
            v_aug = persist.tile([128, ST, HEADS_PER_CORE, HD + 1], bf16)
            nc.vector.memset(v_aug[:, :, :, HD : HD + 1], 1.0)
            # masked exp weights of block (hp0, j0) staged here so its PV
            # (which needs all of v) can run late, in PE slack, instead of
            # forcing every v projection into the first 16 slots.
            stage = persist.tile([128, ST, 1024], bf16)

            # ---- DMA issue (order = priority) ----
            xT_r = xT[:, :].rearrange("(kt p) s -> p kt s", p=128)
            keepT_r = keepT[:, :].rearrange("(i p) s -> p i s", p=128)

            def dma_xT_w(w):
                nc.sync.dma_start(
                    out=xT_sb[:, :, w * 512 : (w + 1) * 512],
                    in_=xT_r[:, :, w * 512 : (w + 1) * 512],
                )

            def dma_keep_j(j, i0, i1, eng):
                eng.dma_start(
                    out=keepT_sb[:, i0:i1, j * 512 : (j + 1) * 512],
                    in_=keepT_r[:, i0:i1, j * 512 : (j + 1) * 512],
                )

            def dma_w(w_sb, w_dram, eng):
                eng.dma_start(
                    out=w_sb,
                    in_=w_dram[:, :].rearrange("(kt p) c -> p kt c", p=128),
                )

            # Two hwdge queues run concurrently; balance payloads against
            # consumption order. The k/q projection chain gates the first
            # exp, so its inputs (wk, wq, biases, xT w0 split across both
            # queues) come first everywhere.
            # ACT queue: wq + bq + xT w0 upper half + the j0 keep block.
            dma_w(wq_sb, wq, nc.scalar)
            nc.scalar.dma_start(out=bq_sb, in_=bq[:, :])
            nc.scalar.dma_start(
                out=xT_sb[:, 4:8, 0:512], in_=xT_r[:, 4:8, 0:512]
            )
            dma_keep_j(0, 0, 8, nc.scalar)
            dma_keep_j(0, 8, 16, nc.scalar)
            # SP queue: wk + bk + xT w0 lower half, remaining xT windows,
            # then the later keep blocks.
            dma_w(wk_sb, wk, nc.sync)
            nc.sync.dma_start(out=bk_sb, in_=bk[:, :])
            nc.sync.dma_start(
                out=xT_sb[:, 0:4, 0:512], in_=xT_r[:, 0:4, 0:512]
            )
            dma_w(wv_sb, wv, nc.sync)
            nc.sync.dma_start(out=bv_sb, in_=bv[:, :])
            dma_xT_w(1)
            dma_xT_w(2)
            dma_xT_w(3)
            dma_keep_j(1, 0, 16, nc.sync)
            dma_keep_j(2, 0, 16, nc.sync)
            dma_keep_j(3, 0, 16, nc.sync)
            # fmt: off

            # ---- PE warm-up: garbage matmuls on the identity tile keep the
            # HAM activity window busy while the first DMAs land, so the
            # first projection runs at 2.4 GHz instead of 1.2.
            warm = proj_ps.tile([128, 512], f32, tag="proj")
            for r in range(20):
                nc.tensor.matmul(
                    warm,
                    lhsT=id_bf[:, 0:128],
                    rhs=id_bf[:, :],
                    start=(r == 0),
                    stop=False,
                    skip_group_check=True,
                )
            # second warm-up stage gated on the wk DMA, bridging the HAM
            # window until the first projection's xT window lands
            for r in range(10):
                nc.tensor.matmul(
                    warm,
                    lhsT=wk_sb[:, 0, 0:128],
                    rhs=id_bf[:, :],
                    start=False,
                    stop=(r == 9),
                    skip_group_check=True,
                )

            # ---- projection groups (each split into 2 filler units so a
            # single unit never exceeds a slot's PE slack) ----
            _proj_state = {}

            def proj_qk_part(which, blk, w, part):
                w_sb, b_sb, dst = (
                    (wq_sb, bq_sb, qT_sb),
                    (wk_sb, bk_sb, kT_sb),
                )[which]
                if part == 0:
                    ps = proj_ps.tile([128, 512], f32, tag="proj")
                    _proj_state[(which, blk, w)] = ps
                else:
                    ps = _proj_state.pop((which, blk, w))
                for kt in range(4 * part, 4 * part + 4):
                    nc.tensor.matmul(
                        ps,
                        lhsT=w_sb[:, kt, blk * 128 : (blk + 1) * 128],
                        rhs=xT_sb[:, kt, w * 512 : (w + 1) * 512],
                        start=(kt == 0),
                        stop=(kt == KT - 1),
                        skip_group_check=True,
                    )
                if part == 1:
                    nc.vector.tensor_scalar_add(
                        out=dst[:, blk, w * 512 : (w + 1) * 512],
                        in0=ps,
                        scalar1=b_sb[:, blk : blk + 1],
                    )

            def proj_qk(which, blk, w):
                proj_qk_part(which, blk, w, 0)
                proj_qk_part(which, blk, w, 1)

            def proj_v_part(st, part):
                if part == 0:
                    psv = proj_ps.tile([128, COLS], f32, tag="proj")
                    _proj_state[("v", st)] = psv
                    nc.tensor.matmul(
                        psv,
                        lhsT=ones_col[:, :],
                        rhs=bv_sb[:, :],
                        start=True,
                        stop=False,
                        skip_group_check=True,
                    )
                else:
                    psv = _proj_state.pop(("v", st))
                for kt in range(4 * part, 4 * part + 4):
                    nc.tensor.matmul(
                        psv,
                        lhsT=xT_sb[:, kt, st * 128 : (st + 1) * 128],
                        rhs=wv_sb[:, kt, :],
                        start=False,
                        stop=(kt == KT - 1),
                        skip_group_check=True,
                    )
                if part == 1:
                    nc.vector.tensor_copy(
                        out=v_aug[:, st, :, 0:HD],
                        in_=psv.rearrange("p (h d) -> p h d", h=HEADS_PER_CORE),
                    )

            # Filler queue: (deadline_slot, thunk), consumed FIFO — the list
            # is constructed in non-decreasing deadline order with each proj
            # group's two halves adjacent (the single-buffer proj psum pool
            # requires a group to finish before the next starts). Fillers
            # run after the slot's QK and before its PV.
            def u_qk(which, blk, w, part):
                return lambda: proj_qk_part(which, blk, w, part)

            def u_v(st, part):
                return lambda: proj_v_part(st, part)

            _pvj0 = {}

            def u_pvj0(e, part):
                def go():
                    if part == 0:
                        psJ = proj_ps.tile([HD + 1, 512], f32, tag="proj")
                        _pvj0[e] = psJ
                    else:
                        psJ = _pvj0[e]
                    for i in range(4 * part, 4 * part + 4):
                        nc.tensor.matmul(
                            psJ,
                            lhsT=v_aug[:, i, e, :],
                            rhs=stage[:, i, e * 512 : (e + 1) * 512],
                            start=(i == 0),
                            stop=(i == ST - 1),
                            skip_group_check=True,
                        )
                    if part == 3:
                        pv_sb = tails.tile(
                            [HD + 1, 512], f32, tag="pvsb", name=f"pvj0sb{e}"
                        )
                        nc.vector.tensor_copy(out=pv_sb, in_=psJ)
                        _pvj0[(e, "sb")] = pv_sb

                return go

            def u_normj0(e):
                def go():
                    norm_head(0, 0, e, _pvj0.pop((e, "sb")))

                return go

            qk_units = lambda dl, which, blk, w: [
                (dl, u_qk(which, blk, w, 0)),
                (dl, u_qk(which, blk, w, 1)),
            ]
            v_units = lambda dl, st: [(dl, u_v(st, 0)), (dl, u_v(st, 1))]
            fillers = []
            fillers += v_units(1, 0) + v_units(2, 1)
            fillers += qk_units(3, 1, 0, 1)  # k blk0 w1 (QK i=4 at slot 4)
            fillers += v_units(4, 2) + v_units(5, 3) + v_units(6, 4)
            fillers += qk_units(7, 1, 0, 2)
            fillers += v_units(8, 5) + v_units(9, 6) + v_units(10, 7)
            fillers += qk_units(11, 1, 0, 3)
            fillers += v_units(12, 8)
            fillers += qk_units(13, 0, 0, 1)  # q blk0 w1 (j1 at slot 16)
            fillers += v_units(14, 9) + v_units(15, 10) + v_units(16, 11)
            fillers += v_units(17, 12) + v_units(18, 13) + v_units(19, 14)
            fillers += v_units(20, 15)
            fillers += qk_units(28, 0, 0, 2)  # q blk0 w2 (due <= 31)
            fillers += qk_units(33, 1, 1, 0)  # k blk1 (due <= 63)
            fillers += qk_units(35, 1, 1, 1)
            fillers += qk_units(37, 1, 1, 2)
            fillers += qk_units(39, 1, 1, 3)
            fillers += qk_units(44, 0, 0, 3)  # q blk0 w3 (due <= 47)
            fillers += qk_units(48, 0, 1, 0)  # q blk1 w0 (due <= 63)
            fillers += qk_units(50, 0, 1, 1)
            fillers += qk_units(52, 0, 1, 2)
            fillers += qk_units(54, 0, 1, 3)
            # deferred PV of (hp0, j0) through the freed proj psum bank,
            # riding the fillerless hp1-era slack
            fillers += [(68 + 2 * p, u_pvj0(0, p)) for p in range(4)]
            fillers += [(76 + 2 * p, u_pvj0(1, p)) for p in range(4)]
            fillers += [(86, u_normj0(0)), (88, u_normj0(1))]

            def norm_head(hp, j, e, pv_sb):
                h = 2 * hp + e
                ob = tails.tile([128, 4, HD], f32, tag="ob")
                tr = tr_ps.tile([128, 4, HD + 1], f32, tag="tr")
                for c in range(4):
                    nc.tensor.transpose(
                        out=tr[:, c, :],
                        in_=pv_sb[:, c * 128 : (c + 1) * 128],
                        identity=identity[0 : HD + 1, 0 : HD + 1],
                    )
                rc = tails.tile([128, 4], f32, tag="rc")
                nc.vector.reciprocal(out=rc, in_=tr[:, :, HD : HD + 1])
                rc_ap = rc[:, :]
                rc_bcast = bass.AP(
                    tensor=rc_ap.tensor,
                    offset=rc_ap.offset,
                    ap=[*rc_ap.ap, [0, HD]],
                )
                nc.vector.tensor_mul(out=ob, in0=tr[:, :, 0:HD], in1=rc_bcast)
                nc.sync.dma_start(
                    out=o[
                        j * 512 : (j + 1) * 512, h * HD : (h + 1) * HD
                    ].rearrange("(c p) d -> p c d", p=128),
                    in_=ob,
                )

            def drain_thunks(hp, j, pvs):
                """Split the block drain into 3 thunks consumed one per
                slot, so drain work never sits ahead of a seam's QK on the
                PE queue."""
                pv_sbs = []

                def evict():
                    for e in range(2):
                        pv_sb = tails.tile(
                            [HD + 1, 512], f32, tag="pvsb", name=f"pv_sb{e}"
                        )
                        nc.vector.tensor_copy(out=pv_sb, in_=pvs[e])
                        pv_sbs.append(pv_sb)

                def norm(e):
                    return lambda: norm_head(hp, j, e, pv_sbs[e])

                return [evict, norm(0), norm(1)]

            # Preamble: first k/q windows so attention starts immediately.
            proj_qk(1, 0, 0)  # k blk0 w0
            proj_qk(0, 0, 0)  # q blk0 w0

            drainq = []
            pvs = None
            for slot in range(2 * NW * ST):
                hp, rem = divmod(slot, NW * ST)
                j, i = divmod(rem, ST)
                # QK + exp + mask first: keeps ACT fed across block seams.
                lgp = big_ps.tile([128, 1024], f32, tag="big")
                for e in range(2):
                    po = e * 64
                    nc.tensor.matmul(
                        lgp[:, e * 512 : (e + 1) * 512],
                        lhsT=kT_sb[po : po + 64, hp, i * 128 : (i + 1) * 128],
                        rhs=qT_sb[po : po + 64, hp, j * 512 : (j + 1) * 512],
                        start=True,
                        stop=True,
                        skip_group_check=True,
                    )
                ex = expw_pool.tile([128, 1024], bf16)
                nc.scalar.activation(
                    out=ex,
                    in_=lgp,
                    func=mybir.ActivationFunctionType.Exp,
                    scale=float(SCALE),
                )
                defer = slot < ST  # (hp0, j0): PV deferred via `stage`
                if defer:
                    ex2 = stage[:, i, :]
                else:
                    ex2 = expw2_pool.tile([128, 1024], bf16)
                k_ap = keepT_sb[:, i, j * 512 : (j + 1) * 512]
                k_bcast = bass.AP(
                    tensor=k_ap.tensor,
                    offset=k_ap.offset,
                    ap=[k_ap.ap[0], [0, 2], *k_ap.ap[1:]],
                )
                nc.vector.tensor_mul(
                    out=ex2.rearrange("p (e n) -> p e n", e=2),
                    in0=ex.rearrange("p (e n) -> p e n", e=2),
                    in1=k_bcast,
                )
                if drainq:
                    drainq.pop(0)()
                while fillers and fillers[0][0] <= slot:
                    fillers.pop(0)[1]()
                if not defer:
                    if i == 0:
                        pvs = [
                            pv_ps.tile(
                                [HD + 1, 512], f32, tag="pv", name=f"pv{e}"
                            )
                            for e in range(2)
                        ]
                    for e in range(2):
                        nc.tensor.matmul(
                            pvs[e],
                            lhsT=v_aug[:, i, 2 * hp + e, :],
                            rhs=ex2[:, e * 512 : (e + 1) * 512],
                            start=(i == 0),
                            stop=(i == ST - 1),
                            skip_group_check=True,
                        )
                    if i == ST - 1:
                        drainq.extend(drain_thunks(hp, j, pvs))
            while drainq:
                drainq.pop(0)()

    _split_multiwait(nc)
    return nc


def _split_multiwait(nc, max_waits: int = 1):
    import concourse.mybir as mybir

    for f in nc.m.functions:
        for blk in f.blocks:
            out = []
            changed = False
            for inst in blk.instructions:
                si = inst.sync_info
                if si is not None and len(si.on_wait) > max_waits:
                    waits = list(si.on_wait)
                    extra = waits[: len(waits) - max_waits]
                    keep = waits[len(waits) - max_waits :]
                    for k, w in enumerate(extra):
                        out.append(
                            mybir.InstNoOp(
                                name=f"{inst.name}-wfx{k}",
                                engine=inst.engine,
                                sync_info=mybir.SyncInfo(on_wait=[w], on_update=[]),
                                bass_nofuse=True,
                            )
                        )
                    inst.sync_info = mybir.SyncInfo(
                        on_wait=keep, on_update=list(si.on_update)
                    )
                    changed = True
                out.append(inst)
            if changed:
                blk.instructions = out
    return nc


def _prep_in_maps(x, mask, Wq, bq, Wk, bk, Wv, bv):
    import ml_dtypes

    bf16 = ml_dtypes.bfloat16
    x = np.asarray(x, np.float32)
    mask = np.asarray(mask, bool)

    xT_b = [np.ascontiguousarray(x[b].T).astype(bf16) for b in range(B)]
    keepT_b = [
        np.ascontiguousarray((~mask[b, 0]).T).astype(bf16) for b in range(B)
    ]
    WqT = np.asarray(Wq, np.float32).T.astype(bf16)
    WkT = np.asarray(Wk, np.float32).T.astype(bf16)
    WvT = np.asarray(Wv, np.float32).T.astype(bf16)
    bq32 = np.asarray(bq, np.float32)
    bk32 = np.asarray(bk, np.float32)
    bv = np.asarray(bv, np.float32).astype(bf16)

    in_maps = []
    for c in range(N_CORES):
        b, g = divmod(c, 4)
        cols = slice(g * COLS, (g + 1) * COLS)
        in_maps.append(
            {
                "xT": xT_b[b],
                "wq": np.ascontiguousarray(WqT[:, cols]),
                "wk": np.ascontiguousarray(WkT[:, cols]),
                "wv": np.ascontiguousarray(WvT[:, cols]),
                "bq": np.ascontiguousarray(bq32[cols].reshape(2, 128).T),
                "bk": np.ascontiguousarray(bk32[cols].reshape(2, 128).T),
                "bv": np.ascontiguousarray(bv[cols].reshape(1, COLS)),
                "keepT": keepT_b[b],
            }
        )
    return in_maps


def kernel(x, mask, Wq, bq, Wk, bk, Wv, bv, _trace=False):
    from concourse.bass_utils import run_bass_kernel_spmd

    if "nc" not in _cache:
        _cache["nc"] = _build_nc()
    nc = _cache["nc"]

    in_maps = _prep_in_maps(x, mask, Wq, bq, Wk, bk, Wv, bv)
    res = run_bass_kernel_spmd(
        nc, in_maps, core_ids=list(range(N_CORES)), trace=_trace
    )
    _cache["last_result"] = res

    out = np.empty((B, S, D), np.float32)
    for c in range(N_CORES):
        b, g = divmod(c, 4)
        out[b, :, g * COLS : (g + 1) * COLS] = res.results[c]["o"]
    return out
